# revision 25
# baseline (speedup 1.0000x reference)
"""DigitCaps (dead-code-routing collapsed) Trainium2 Bass kernel.

Math (faithful to the reference):
    s[j,d]  = (1/512) * sum_{i,k} W[0,i,j,d,k] * x[i,k]      (10,16)
    sq      = s^2                                             (elementwise; last axis is size 1)
    out     = (sq/(1+sq)) * s/(sqrt(sq+EPS)+EPS)              (1,1,10,16,1)

Sharding: the 16-wide output dim `d` is split across 8 cores (2 each). Each
core reads its own 1/8 slice of W (320 KB) and computes its 20 outputs fully;
no cross-core reduction is needed. Host-side work is only slicing/packing of
inputs and concatenation of the 8 disjoint output slices.

Per-core device program (SPMD, identical on all cores):
    input is packed as S blocks [x_s | W_s] and fetched with S DMAs that
    alternate between the two HWDGE rings (SP engine / ACT engine) so the
    premultiply of block s overlaps the transfer of block s+1:
        W_s laid out [p, (t', n, k)]: contraction q=(i,k), i = t*128 + p,
        n = j*2+dd, t = s*CPS + t'
    DVE: T[p,t,n,k] = W[p,t,n,k] * x[p,t,k]  (stride-0 broadcast over n,
         one tensor_tensor per block)
    PE:  4 accumulating float32r matmuls (a 1/512 column as the stationary
         operand reduces partitions; f32r keeps the fp32 matmul single-pass)
         -> psum[1, (n, k)]
    DVE: reduce over k -> s[1, 20]; squash chain (sqrt on ACT); DMA out.
"""

import os
import sys
from contextlib import ExitStack

import numpy as np

for _p in ("/opt/trn_rl_repo", "/root/.axon_site/_ro/trn_rl_repo"):
    if os.path.isdir(_p) and _p not in sys.path:
        sys.path.append(_p)

N_IN, N_OUT, D_IN, D_OUT = 512, 10, 8, 16
EPS = 1e-7
N_CORES = 8
D_PER = D_OUT // N_CORES          # 2 output dims per core
N_PER = N_OUT * D_PER             # 20 outputs per core
P = 128                           # partitions
T = N_IN // P                     # 4 i-chunks of 128
K = D_IN                          # 8
CW = N_PER * K                    # 160 W cols per chunk

S = int(os.environ.get("DIGITCAPS_SPLIT", "4"))   # DMA/premult pipeline depth
assert T % S == 0 or S % T == 0
CPS = max(T // S, 1)              # chunks per split block
XB = CPS * K                      # x cols per block
WB = CPS * CW                     # W cols per block
BLK = XB + WB
TOT = S * BLK

USE_F32R = os.environ.get("DIGITCAPS_F32R", "1") == "1"

_built = None
last_results = None               # BassKernelResults of the most recent run


def _new_nc():
    """Bacc instance with the (dead, for this kernel) init-time const-AP
    memsets skipped — they sit on GpSimd before the init all-engine barrier
    and delay the first DMA."""
    import concourse.bass as bass
    from concourse import bacc

    if os.environ.get("DIGITCAPS_SKIP_CONST_MEMSET", "1") != "1":
        return bacc.Bacc("TRN2", num_devices=N_CORES)
    probe = bass.BassEitherVectorEngine
    orig = probe.memset
    probe.memset = lambda self, ap, constant: None
    try:
        nc = bacc.Bacc("TRN2", num_devices=N_CORES)
    finally:
        probe.memset = orig
    return nc


def _build_nc():
    import concourse.bass as bass
    import concourse.tile as tile
    from concourse import mybir

    nc = _new_nc()
    inp = nc.dram_tensor("inp", (P, TOT), mybir.dt.float32, kind="ExternalInput")
    out = nc.dram_tensor("out", (1, N_PER), mybir.dt.float32, kind="ExternalOutput")

    f32 = mybir.dt.float32
    f32r = mybir.dt.float32r
    with tile.TileContext(nc) as tc, ExitStack() as ctx:
        pool = ctx.enter_context(tc.tile_pool(name="p", bufs=1))
        pspool = ctx.enter_context(tc.tile_pool(name="ps", bufs=1, space="PSUM"))

        buf = pool.tile([P, TOT], f32)
        for s_i in range(S):
            eng = nc.sync if s_i % 2 == 0 else nc.scalar
            eng.dma_start(
                out=buf[:, s_i * BLK : (s_i + 1) * BLK],
                in_=inp[:, s_i * BLK : (s_i + 1) * BLK],
            )

        # stationary 1/512 column; written on DVE so the matmul's lhsT and
        # rhs deps ride one semaphore (walrus fits one wait per compute op).
        # f32r producers must "round to f32r", hence memset+copy.
        ones = pool.tile([P, 1], f32)
        if USE_F32R:
            ones_raw = pool.tile([P, 1], f32)
            nc.vector.memset(ones_raw, 1.0 / N_IN)
            nc.vector.tensor_copy(ones.bitcast(f32r), ones_raw)
        else:
            nc.vector.memset(ones, 1.0 / N_IN)

        # T[p, t', n, k] = W[p, t', n, k] * x[p, t', k]; one TT per block
        tmul = pool.tile([P, T * CW], f32)
        for s_i in range(S):
            x_sl = buf[:, s_i * BLK : s_i * BLK + XB]
            x_b = bass.AP(
                tensor=x_sl.tensor,
                offset=x_sl.offset,
                ap=[x_sl.ap[0], [K, CPS], [0, N_PER], [1, K]],
            )
            w_4d = buf[
                :, s_i * BLK + XB : (s_i + 1) * BLK
            ].rearrange("p (t n k) -> p t n k", t=CPS, n=N_PER)
            t_4d = tmul[:, s_i * WB : (s_i + 1) * WB].rearrange(
                "p (t n k) -> p t n k", t=CPS, n=N_PER
            )
            if USE_F32R:
                t_4d = t_4d.bitcast(f32r)
            nc.vector.tensor_tensor(t_4d, w_4d, x_b, op=mybir.AluOpType.mult)

        # psum[0, (n, k)] = (1/512) * sum_{p, t} T[p, t, n, k]
        ps = pspool.tile([1, CW], f32)
        for t in range(T):
            lhsT = ones[:, 0:1]
            rhs = tmul[:, t * CW : (t + 1) * CW]
            if USE_F32R:
                lhsT = lhsT.bitcast(f32r)
                rhs = rhs.bitcast(f32r)
            nc.tensor.matmul(
                ps[0:1, :], lhsT=lhsT, rhs=rhs, start=(t == 0), stop=(t == T - 1)
            )

        # s[1, n] = sum_k psum[1, (n, k)]
        s = pool.tile([1, N_PER], f32)
        nc.vector.tensor_reduce(
            s,
            ps[0:1, :].rearrange("p (n k) -> p n k", n=N_PER),
            axis=mybir.AxisListType.X,
            op=mybir.AluOpType.add,
        )

        # squash: out = (s*sq) / ((1+sq) * (sqrt(sq+EPS)+EPS))
        # sq on DVE (not ACT) so no op needs waits on two different sems.
        eps_t = pool.tile([1, 1], f32)
        nc.vector.memset(eps_t, EPS)
        sq = pool.tile([1, N_PER], f32)
        nc.vector.tensor_mul(sq, s, s)
        r = pool.tile([1, N_PER], f32)
        nc.scalar.activation(
            r, sq, mybir.ActivationFunctionType.Sqrt, bias=eps_t[0:1, 0:1]
        )
        num = pool.tile([1, N_PER], f32)
        nc.vector.tensor_mul(num, s, sq)
        d1 = pool.tile([1, N_PER], f32)
        nc.vector.tensor_scalar_add(d1, sq, 1.0)
        d2 = pool.tile([1, N_PER], f32)
        nc.vector.tensor_scalar_add(d2, r, EPS)
        den = pool.tile([1, N_PER], f32)
        nc.vector.tensor_mul(den, d1, d2)
        rec = pool.tile([1, N_PER], f32)
        nc.vector.reciprocal(rec, den)
        q = pool.tile([1, N_PER], f32)
        nc.vector.tensor_mul(q, num, rec)

        nc.sync.dma_start(out=out[:, :], in_=q)
    nc.finalize()
    return nc


def kernel(x, W):
    global _built, last_results
    from concourse.bass_utils import run_bass_kernel_spmd

    if _built is None:
        _built = _build_nc()
    nc = _built

    x = np.ascontiguousarray(np.asarray(x, dtype=np.float32))
    W = np.ascontiguousarray(np.asarray(W, dtype=np.float32))

    # xr[p, t*K + k] = x[t*128 + p, k]
    xr = x.reshape(T, P, K).transpose(1, 0, 2).reshape(P, T * K)
    base = np.empty((P, TOT), dtype=np.float32)
    for s_i in range(S):
        base[:, s_i * BLK : s_i * BLK + XB] = xr[
            :, s_i * CPS * K : (s_i + 1) * CPS * K
        ]

    in_maps = []
    for c in range(N_CORES):
        Wc = W[0][:, :, D_PER * c : D_PER * (c + 1), :]     # (512, 10, 2, 8)
        Wr = (
            Wc.reshape(T, P, N_OUT, D_PER, K)
            .transpose(1, 0, 2, 3, 4)
            .reshape(P, T * CW)
        )
        buf = base.copy()
        for s_i in range(S):
            buf[:, s_i * BLK + XB : (s_i + 1) * BLK] = Wr[
                :, s_i * WB : (s_i + 1) * WB
            ]
        in_maps.append({"inp": buf})

    res = run_bass_kernel_spmd(nc, in_maps, core_ids=list(range(N_CORES)))
    last_results = res

    v = np.zeros((N_OUT, D_OUT), dtype=np.float32)
    for c in range(N_CORES):
        v[:, D_PER * c : D_PER * (c + 1)] = res.results[c]["out"].reshape(
            N_OUT, D_PER
        )
    return v.reshape(1, 1, N_OUT, D_OUT, 1)


# revision 34
# speedup vs baseline: 1.0257x; 1.0257x over previous
"""DigitCaps (dead-code-routing collapsed) Trainium2 Bass kernel.

Math (faithful to the reference):
    s[j,d]  = (1/512) * sum_{i,k} W[0,i,j,d,k] * x[i,k]      (10,16)
    sq      = s^2                                             (elementwise; last axis is size 1)
    out     = (sq/(1+sq)) * s/(sqrt(sq+EPS)+EPS)              (1,1,10,16,1)

Sharding: the 16-wide output dim `d` is split across 8 cores (2 each). Each
core reads its own 1/8 slice of W (320 KB) and computes its 20 outputs fully;
no cross-core reduction is needed. Host-side work is only slicing/packing of
inputs and concatenation of the 8 disjoint output slices.

Per-core device program (SPMD, identical on all cores):
    input is packed as S blocks [x_s | W_s] and fetched with S DMAs that
    alternate between the two HWDGE rings (SP engine / ACT engine) so the
    premultiply of block s overlaps the transfer of block s+1:
        W_s laid out [p, (t', n, k)]: contraction q=(i,k), i = t*128 + p,
        n = j*2+dd, t = s*CPS + t'
    DVE: T[p,t,n,k] = W[p,t,n,k] * x[p,t,k]  (stride-0 broadcast over n,
         one tensor_tensor per block)
    PE:  4 accumulating float32r matmuls (a 1/512 column as the stationary
         operand reduces partitions; f32r keeps the fp32 matmul single-pass)
         -> psum[1, (n, k)]
    DVE: reduce over k -> s[1, 20]; squash chain (sqrt on ACT); DMA out.
"""

import os
import sys
from contextlib import ExitStack

import numpy as np

for _p in ("/opt/trn_rl_repo", "/root/.axon_site/_ro/trn_rl_repo"):
    if os.path.isdir(_p) and _p not in sys.path:
        sys.path.append(_p)

N_IN, N_OUT, D_IN, D_OUT = 512, 10, 8, 16
EPS = 1e-7
N_CORES = 8
D_PER = D_OUT // N_CORES          # 2 output dims per core
N_PER = N_OUT * D_PER             # 20 outputs per core
P = 128                           # partitions
T = N_IN // P                     # 4 i-chunks of 128
K = D_IN                          # 8
CW = N_PER * K                    # 160 W cols per chunk

# DMA/premult pipeline: chunk-counts per block, e.g. "2,2" or "3,1"
BLOCKS = [
    int(b) for b in os.environ.get("DIGITCAPS_BLOCKS", "2,2").split(",")
]
assert sum(BLOCKS) == T
S = len(BLOCKS)
_off = [0]
for _b in BLOCKS:
    _off.append(_off[-1] + _b * (K + CW))
BLK_OFF = _off                    # column offset of each block
TOT = BLK_OFF[-1]

USE_F32R = os.environ.get("DIGITCAPS_F32R", "1") == "1"

_built = None
last_results = None               # BassKernelResults of the most recent run


def _ensure_ntff_hook_module():
    """bass_utils imports antenv.axon_hooks when BASS_TRACE is set; that
    module is absent in some containers. Register a functional stand-in
    (real ctypes NTFF hook when libaxon + trn_boot are present, else a
    None-returning stub so tracing degrades to a warning)."""
    import types

    try:
        import antenv  # noqa: F401
    except ImportError:
        return
    try:
        import antenv.axon_hooks  # noqa: F401
        return
    except ImportError:
        pass
    hook = None
    boot_dir = "/root/.axon_site/trn_agent_boot"
    so = "/opt/axon/libaxon_pjrt.so"
    if os.path.isdir(boot_dir) and os.path.exists(so):
        if boot_dir not in sys.path:
            sys.path.append(boot_dir)
        try:
            import trn_boot

            hook = trn_boot._ntff_profile_via_ctypes(so)
        except Exception:
            hook = None
    mod = types.ModuleType("antenv.axon_hooks")
    mod._hook = hook
    mod.get_axon_ntff_profile_hook = lambda: mod._hook
    mod.set_axon_ntff_profile_hook = lambda h: setattr(mod, "_hook", h)
    sys.modules["antenv.axon_hooks"] = mod
    import antenv as _a

    _a.axon_hooks = mod


def _new_nc():
    """Bacc instance with the (dead, for this kernel) init-time const-AP
    memsets skipped — they sit on GpSimd before the init all-engine barrier
    and delay the first DMA."""
    import concourse.bass as bass
    from concourse import bacc

    if os.environ.get("DIGITCAPS_SKIP_CONST_MEMSET", "1") != "1":
        return bacc.Bacc("TRN2", num_devices=N_CORES)
    probe = bass.BassEitherVectorEngine
    orig = probe.memset
    probe.memset = lambda self, ap, constant: None
    try:
        nc = bacc.Bacc("TRN2", num_devices=N_CORES)
    finally:
        probe.memset = orig
    return nc


def _build_nc():
    import concourse.bass as bass
    import concourse.tile as tile
    from concourse import mybir

    nc = _new_nc()
    inp = nc.dram_tensor("inp", (P, TOT), mybir.dt.float32, kind="ExternalInput")
    out = nc.dram_tensor("out", (1, N_PER), mybir.dt.float32, kind="ExternalOutput")

    f32 = mybir.dt.float32
    f32r = mybir.dt.float32r
    with tile.TileContext(nc) as tc, ExitStack() as ctx:
        pool = ctx.enter_context(tc.tile_pool(name="p", bufs=1))
        pspool = ctx.enter_context(tc.tile_pool(name="ps", bufs=1, space="PSUM"))

        buf = pool.tile([P, TOT], f32)
        for s_i in range(S):
            eng = nc.sync if s_i % 2 == 0 else nc.scalar
            eng.dma_start(
                out=buf[:, BLK_OFF[s_i] : BLK_OFF[s_i + 1]],
                in_=inp[:, BLK_OFF[s_i] : BLK_OFF[s_i + 1]],
            )

        # stationary 1/512 column; written on DVE so the matmul's lhsT and
        # rhs deps ride one semaphore (walrus fits one wait per compute op).
        # f32r producers must "round to f32r", hence memset+copy.
        ones = pool.tile([P, 1], f32)
        if USE_F32R:
            ones_raw = pool.tile([P, 1], f32)
            nc.vector.memset(ones_raw, 1.0 / N_IN)
            nc.vector.tensor_copy(ones.bitcast(f32r), ones_raw)
        else:
            nc.vector.memset(ones, 1.0 / N_IN)

        n_warm = int(os.environ.get("DIGITCAPS_WARMUP_MM", "0"))
        if n_warm:
            # Dummy matmuls during the DMA window keep the PE busy so the HAM
            # clock gate lifts (1.2 -> 2.4 GHz) before the real matmuls.
            warm_w = pool.tile([P, 1], f32)
            nc.vector.memset(warm_w, 1.0)
            warm_rhs = pool.tile([P, 512], f32)
            nc.vector.memset(warm_rhs, 1.0)
            warm_ps = pspool.tile([1, 512], f32)
            for _ in range(n_warm):
                nc.tensor.matmul(
                    warm_ps[0:1, :], lhsT=warm_w[:, 0:1], rhs=warm_rhs,
                    start=True, stop=True,
                )

        # T[p, t', n, k] = W[p, t', n, k] * x[p, t', k]; one TT per block
        tmul = pool.tile([P, T * CW], f32)
        for s_i in range(S):
            nb = BLOCKS[s_i]
            cs = sum(BLOCKS[:s_i])
            x_lo = BLK_OFF[s_i]
            w_lo = x_lo + nb * K
            x_sl = buf[:, x_lo : x_lo + nb * K]
            x_b = bass.AP(
                tensor=x_sl.tensor,
                offset=x_sl.offset,
                ap=[x_sl.ap[0], [K, nb], [0, N_PER], [1, K]],
            )
            w_4d = buf[:, w_lo : BLK_OFF[s_i + 1]].rearrange(
                "p (t n k) -> p t n k", t=nb, n=N_PER
            )
            t_4d = tmul[:, cs * CW : (cs + nb) * CW].rearrange(
                "p (t n k) -> p t n k", t=nb, n=N_PER
            )
            if USE_F32R:
                t_4d = t_4d.bitcast(f32r)
            nc.vector.tensor_tensor(t_4d, w_4d, x_b, op=mybir.AluOpType.mult)

        # psum[0, (n, k)] = (1/512) * sum_{p, t} T[p, t, n, k]
        ps = pspool.tile([1, CW], f32)
        for t in range(T):
            lhsT = ones[:, 0:1]
            rhs = tmul[:, t * CW : (t + 1) * CW]
            if USE_F32R:
                lhsT = lhsT.bitcast(f32r)
                rhs = rhs.bitcast(f32r)
            nc.tensor.matmul(
                ps[0:1, :], lhsT=lhsT, rhs=rhs, start=(t == 0), stop=(t == T - 1)
            )

        if os.environ.get("DIGITCAPS_TSQUASH", "0") == "1":
            # Column-form squash: flip s onto 20 partitions with a DVE 32x32
            # block transpose so every squash op pays FD=1 cost, then flip the
            # result back for a contiguous output DMA.
            SQ = 32
            t_in = pool.tile([SQ, SQ], f32)
            nc.vector.memset(t_in, 0.0)
            eps_t = pool.tile([SQ, 1], f32)
            nc.vector.memset(eps_t, EPS)
            # s -> row 0 of t_in
            nc.vector.tensor_reduce(
                t_in[0:1, 0:N_PER],
                ps[0:1, :].rearrange("p (n k) -> p n k", n=N_PER),
                axis=mybir.AxisListType.X,
                op=mybir.AluOpType.add,
            )
            t_sc = pool.tile([SQ, SQ], f32)
            nc.vector.transpose(t_sc, t_in)
            s_c = t_sc[0:N_PER, 0:1]
            sq = pool.tile([SQ, 1], f32)
            nc.vector.tensor_mul(sq[0:N_PER], s_c, s_c)
            r = pool.tile([SQ, 1], f32)
            nc.scalar.activation(
                r[0:N_PER],
                sq[0:N_PER],
                mybir.ActivationFunctionType.Sqrt,
                bias=eps_t[0:N_PER],
            )
            num = pool.tile([SQ, 1], f32)
            nc.vector.tensor_mul(num[0:N_PER], s_c, sq[0:N_PER])
            d1 = pool.tile([SQ, 1], f32)
            nc.vector.tensor_scalar_add(d1[0:N_PER], sq[0:N_PER], 1.0)
            d2 = pool.tile([SQ, 1], f32)
            nc.vector.tensor_scalar_add(d2[0:N_PER], r[0:N_PER], EPS)
            den = pool.tile([SQ, 1], f32)
            nc.vector.tensor_mul(den[0:N_PER], d1[0:N_PER], d2[0:N_PER])
            rec = pool.tile([SQ, 1], f32)
            nc.vector.reciprocal(rec[0:N_PER], den[0:N_PER])
            t_out = pool.tile([SQ, SQ], f32)
            nc.vector.memset(t_out, 0.0)
            nc.vector.tensor_mul(t_out[0:N_PER, 0:1], num[0:N_PER], rec[0:N_PER])
            t_fin = pool.tile([SQ, SQ], f32)
            nc.vector.transpose(t_fin, t_out)
            nc.sync.dma_start(out=out[:, :], in_=t_fin[0:1, 0:N_PER])
        else:
            # s[1, n] = sum_k psum[1, (n, k)]
            s = pool.tile([1, N_PER], f32)
            nc.vector.tensor_reduce(
                s,
                ps[0:1, :].rearrange("p (n k) -> p n k", n=N_PER),
                axis=mybir.AxisListType.X,
                op=mybir.AluOpType.add,
            )

            # squash: out = s*sq * [1/(1+sq)] * [1/(sqrt(sq+EPS)+EPS)]
            # The (1+sq) reciprocal and s*sq products run on DVE while ACT
            # computes the sqrt, so the post-sqrt critical path is only
            # (+EPS) -> reciprocal -> final multiply.
            # sq on DVE (not ACT) so no op needs waits on two different sems.
            eps_t = pool.tile([1, 1], f32)
            nc.vector.memset(eps_t, EPS)
            sq = pool.tile([1, N_PER], f32)
            nc.vector.tensor_mul(sq, s, s)
            r = pool.tile([1, N_PER], f32)
            nc.scalar.activation(
                r, sq, mybir.ActivationFunctionType.Sqrt, bias=eps_t[0:1, 0:1]
            )
            # hidden under the ACT sqrt:
            num = pool.tile([1, N_PER], f32)
            nc.vector.tensor_mul(num, s, sq)
            d1 = pool.tile([1, N_PER], f32)
            nc.vector.tensor_scalar_add(d1, sq, 1.0)
            rec1 = pool.tile([1, N_PER], f32)
            nc.vector.reciprocal(rec1, d1)
            m1 = pool.tile([1, N_PER], f32)
            nc.vector.tensor_mul(m1, num, rec1)
            # post-sqrt path:
            d2 = pool.tile([1, N_PER], f32)
            nc.vector.tensor_scalar_add(d2, r, EPS)
            rec2 = pool.tile([1, N_PER], f32)
            nc.vector.reciprocal(rec2, d2)
            q = pool.tile([1, N_PER], f32)
            nc.vector.tensor_mul(q, m1, rec2)

            nc.sync.dma_start(out=out[:, :], in_=q)
    nc.finalize()
    return nc


def kernel(x, W):
    global _built, last_results
    _ensure_ntff_hook_module()
    from concourse.bass_utils import run_bass_kernel_spmd

    if _built is None:
        _built = _build_nc()
    nc = _built

    x = np.ascontiguousarray(np.asarray(x, dtype=np.float32))
    W = np.ascontiguousarray(np.asarray(W, dtype=np.float32))

    # xr[p, t*K + k] = x[t*128 + p, k]
    xr = x.reshape(T, P, K).transpose(1, 0, 2).reshape(P, T * K)
    base = np.empty((P, TOT), dtype=np.float32)
    for s_i in range(S):
        nb, cs = BLOCKS[s_i], sum(BLOCKS[:s_i])
        base[:, BLK_OFF[s_i] : BLK_OFF[s_i] + nb * K] = xr[
            :, cs * K : (cs + nb) * K
        ]

    in_maps = []
    for c in range(N_CORES):
        Wc = W[0][:, :, D_PER * c : D_PER * (c + 1), :]     # (512, 10, 2, 8)
        Wr = (
            Wc.reshape(T, P, N_OUT, D_PER, K)
            .transpose(1, 0, 2, 3, 4)
            .reshape(P, T * CW)
        )
        buf = base.copy()
        for s_i in range(S):
            nb, cs = BLOCKS[s_i], sum(BLOCKS[:s_i])
            buf[:, BLK_OFF[s_i] + nb * K : BLK_OFF[s_i + 1]] = Wr[
                :, cs * CW : (cs + nb) * CW
            ]
        in_maps.append({"inp": buf})

    res = run_bass_kernel_spmd(nc, in_maps, core_ids=list(range(N_CORES)))
    last_results = res

    v = np.zeros((N_OUT, D_OUT), dtype=np.float32)
    for c in range(N_CORES):
        v[:, D_PER * c : D_PER * (c + 1)] = res.results[c]["out"].reshape(
            N_OUT, D_PER
        )
    return v.reshape(1, 1, N_OUT, D_OUT, 1)
